# revision 52
# baseline (speedup 1.0000x reference)
"""MixerGatedDeltaNet TRN2 kernel: full-input entry point.

kernel(**inputs) -> np.ndarray [4, 4096, 16, 128] float32.

Sharding: 8 NeuronCores = 4 batches x 2 head-groups. Each core runs the same
Bass program (SPMD) on its (batch, head-group) shard; outputs are gathered.
"""
import math
import sys
from contextlib import ExitStack

import numpy as np

for p in ("/opt/trn_rl_repo",):
    if p not in sys.path:
        sys.path.insert(0, p)

import ml_dtypes
import concourse.bass as bass
import concourse.bacc as bacc
import concourse.tile as tile
from concourse import mybir
from concourse.bass_utils import run_bass_kernel_spmd

dt = mybir.dt
AF = mybir.ActivationFunctionType
ALU = mybir.AluOpType

# Model dims (per core)
D = 1024
NH = 8            # heads per core
DK = 64
DV = 128
QK_CH = NH * DK   # 512
V_CH = NH * DV    # 1024
IN_COLS = 2 * QK_CH + V_CH + 40  # 2088: q 512 | k 512 | v 1024 | b@0:8,a@32:40
EPS = 1e-6
T_FULL = 4096
TS = 512          # super-chunk (projection granularity)
C = 128           # delta-rule chunk length

F32, BF16, F32R = dt.float32, dt.bfloat16, dt.float32r

_CACHE = {}


def _build(T=T_FULL):
    n_super = T // TS
    ncps = TS // C
    n_levels = int(math.log2(C))

    nc = bacc.Bacc("TRN2", target_bir_lowering=False, debug=False, num_devices=8)

    x_d = nc.dram_tensor("x", [T, D], BF16, kind="ExternalInput").ap()
    wqkv_d = nc.dram_tensor("wqkv", [8, 128, IN_COLS], BF16, kind="ExternalInput").ap()
    wg_d = nc.dram_tensor("wg", [8, 128, V_CH], BF16, kind="ExternalInput").ap()
    cw_d = nc.dram_tensor("cw", [128, 16, 4], F32, kind="ExternalInput").ap()
    smallc_d = nc.dram_tensor("smallc", [8, 4], F32, kind="ExternalInput").ap()
    sel_d = nc.dram_tensor("sel", [8, 8, 128], F32, kind="ExternalInput").ap()
    out_d = nc.dram_tensor("out", [T, V_CH], BF16, kind="ExternalOutput").ap()

    with tile.TileContext(nc) as tc, ExitStack() as ctx:
        P = lambda name, bufs, space="SBUF": ctx.enter_context(
            tc.tile_pool(name=name, bufs=bufs, space=space))

        wpool = P("wpool", 1)
        const_pool = P("const", 1)
        xtpool = P("xt", 2)
        qkpool = P("qk", 2)
        vpool = P("v", 2)
        gatepool = P("gate", 6)
        convpool = P("conv", 1)
        halopool = P("halo", 1)
        rowpool = P("row", 2)
        chpool = P("ch", 3)
        stackpool = P("stack", 4)
        bmpool = P("bm", 12)
        upool = P("u", 12)
        wppool = P("wp", 8)
        tqspool = P("tqs", 9)
        scrpool = P("scr", 2)
        wptpool = P("wpt", 3)
        row1pool = P("row1", 1)
        opool = P("o", 2)
        state_pool = P("state", 1)
        ps_proj = P("ps_proj", 2, "PSUM")
        ps_scan = P("ps_scan", 4, "PSUM")
        ps_b = P("ps_b", 2, "PSUM")

        wqkv_s = wpool.tile([128, 8, IN_COLS], BF16)
        nc.sync.dma_start(wqkv_s[:], wqkv_d.rearrange("k p c -> p k c"))
        wg_s = wpool.tile([128, 8, V_CH], BF16)
        nc.sync.dma_start(wg_s[:], wg_d.rearrange("k p c -> p k c"))
        cw_s = const_pool.tile([128, 16, 4], F32)
        nc.sync.dma_start(cw_s[:], cw_d[:])
        smallc_s = const_pool.tile([8, 4], F32)
        nc.sync.dma_start(smallc_s[:], smallc_d[:])
        sel_s = const_pool.tile([8, 8, 128], F32)
        nc.sync.dma_start(sel_s[:], sel_d[:])
        dtb_col = smallc_s[:, 0:1]
        nA_col = smallc_s[:, 1:2]

        identf = const_pool.tile([128, 128], F32)
        ident = const_pool.tile([128, 128], BF16)
        onesf = const_pool.tile([128, 128], F32)
        onesbd = const_pool.tile([128, 2], BF16)
        zeros8 = const_pool.tile([8, C], F32)
        epsc = const_pool.tile([48, 1], F32)
        nc.vector.memset(onesf[:], 1.0)
        nc.vector.memset(zeros8[:], 0.0)
        nc.vector.memset(epsc[:], EPS)
        nc.gpsimd.affine_select(identf[:], onesf[:], pattern=[[-1, 128]],
                                compare_op=ALU.is_equal, fill=0.0, base=0,
                                channel_multiplier=1)
        nc.vector.tensor_copy(ident[:], identf[:])
        nc.vector.memset(onesbd[:], 0.0)
        nc.vector.memset(onesbd[0:64, 0:1], 1.0)
        nc.vector.memset(onesbd[64:128, 1:2], 1.0)

        # per-ti selector stationaries: half-partition sums routed to rows
        # 2ti, 2ti+1 of a shared [16, TS] accumulation bank
        onesbd_all = const_pool.tile([128, 8, 8], BF16)
        nc.vector.memset(onesbd_all[:], 0.0)
        for ti in range(8):
            base = 2 * (ti % 4)
            nc.vector.memset(onesbd_all[0:64, ti, base:base + 1], 1.0)
            nc.vector.memset(onesbd_all[64:128, ti, base + 1:base + 2], 1.0)

        S_a = state_pool.tile([128, 4, DV], BF16, tag="Sa")
        S_b = state_pool.tile([128, 4, DV], BF16, tag="Sb")
        S_tiles = [S_a, S_b]
        nc.vector.memset(S_tiles[0][:], 0.0)
        nc.vector.memset(S_tiles[1][:], 0.0)

        def s_slice(S, h):
            lo = (h % 2) * 64
            return S[lo:lo + 64, h // 2, :]

        diag_cw = wpool.tile([128, 16, 4, 128], BF16)
        for ct in range(16):
            for i in range(4):
                nc.vector.tensor_scalar_mul(diag_cw[:, ct, i, :], ident[:],
                                            cw_s[:, ct, i:i + 1])

        halo = halopool.tile([128, 16, 3], BF16)
        nc.vector.memset(halo[:], 0.0)

        chunk_state = {"idx": 0}

        def gen_proj(s, sc):
            t0 = s * TS
            xt = xtpool.tile([128, 8, TS], BF16)
            nc.sync.dma_start_transpose(xt[:], x_d[t0:t0 + TS, :])
            yield
            xtr = xt[:]

            qkT = qkpool.tile([128, 8, TS], BF16)
            vT = vpool.tile([128, 8, TS], BF16)
            t_beta = rowpool.tile([8, TS], F32, tag="beta")
            t_g = row1pool.tile([8, TS], F32, tag="g")
            t_gc = rowpool.tile([8, TS], F32, tag="gc")
            t_lnb = row1pool.tile([8, TS], F32, tag="lnb")
            t_lnq = row1pool.tile([8, TS], F32, tag="lnq")
            t_lnk = row1pool.tile([8, TS], F32, tag="lnk")
            t_avmv = rowpool.tile([8, 2, TS], F32, tag="avmv")
            t_rv = rowpool.tile([8, TS], F32, tag="rv")
            sc.update(t0=t0, qkT=qkT, vT=vT, t_beta=t_beta, t_gc=t_gc,
                      t_avmv=t_avmv, t_rv=t_rv)

            for ct in range(17):
                c_lo = ct * 128
                n_cols = 128 if ct < 16 else 40
                psp = ps_proj.tile([128, 512], F32, tag="psp")
                for kt in range(8):
                    nc.tensor.matmul(psp[0:n_cols, :],
                                     wqkv_s[:, kt, c_lo:c_lo + n_cols],
                                     xtr[:, kt, :],
                                     start=(kt == 0), stop=(kt == 7))
                if ct < 16:
                    buf = convpool.tile([128, 3 + TS], BF16, tag="cbuf")
                    nc.vector.tensor_copy(buf[:, 0:3], halo[:, ct, :])
                    nc.any.tensor_copy(buf[:, 3:3 + TS], psp[:])
                    nc.vector.tensor_copy(halo[:, ct, :], buf[:, TS:TS + 3])
                    ps_c = ps_proj.tile([128, 512], F32, tag="psp")
                    for i in range(4):
                        nc.tensor.matmul(ps_c[:], diag_cw[:, ct, i, :],
                                         buf[:, i:i + TS],
                                         start=(i == 0), stop=(i == 3))
                    dst = qkT[:, ct, :] if ct < 8 else vT[:, ct - 8, :]
                    nc.scalar.activation(dst, ps_c[:], AF.Silu)
                else:
                    # beta = 1/(1+e^-b); t_lnb = ln(1+e^-b) = -ln(beta)
                    # g = nA * ln(1+e^(a+dtb))   (nA = -exp(A_log))
                    e3 = row1pool.tile([8, TS], F32, tag="e3")
                    nc.scalar.activation(e3[:], psp[0:8, :], AF.Exp, scale=-1.0)
                    nc.vector.tensor_scalar_add(e3[:], e3[:], 1.0)
                    nc.vector.reciprocal(t_beta[:], e3[:])
                    nc.scalar.activation(t_lnb[:], e3[:], AF.Ln)
                    e2 = row1pool.tile([8, TS], F32, tag="e2")
                    nc.scalar.activation(e2[:], psp[32:40, :], AF.Exp, bias=dtb_col)
                    nc.vector.tensor_scalar_add(e2[:], e2[:], 1.0)
                    nc.scalar.activation(e2[:], e2[:], AF.Ln)
                    nc.vector.tensor_scalar_mul(t_g[:], e2[:], nA_col)
                yield
            for cc in range(ncps):
                nc.vector.tensor_tensor_scan(t_gc[:, cc * C:(cc + 1) * C],
                                             t_g[:, cc * C:(cc + 1) * C],
                                             zeros8[:], 0.0, ALU.add, ALU.add)

            psn_q = ps_scan.tile([8, TS], F32, tag="ps")
            psn_k = ps_scan.tile([8, TS], F32, tag="ps")
            for ti in range(8):
                sq = scrpool.tile([128, TS], BF16, tag="sq")
                nc.gpsimd.tensor_mul(sq[:], qkT[:, ti, :], qkT[:, ti, :])
                psn = psn_q if ti < 4 else psn_k
                nc.tensor.matmul(psn[:], onesbd_all[:, ti, :], sq[:],
                                 start=(ti % 4 == 0), stop=(ti % 4 == 3),
                                 skip_group_check=True)
            nc.scalar.activation(t_lnq[:], psn_q[:], AF.Ln, bias=epsc[0:8, :])
            nc.scalar.activation(t_lnk[:], psn_k[:], AF.Ln, bias=epsc[0:8, :])
            yield

            # av = gc - lnb_pos - 0.5*lnk2 ; mv = gc - 0.5*lnq2 - 0.5*ln(DK)
            # rv = gc + 0.5*lnk2
            nc.vector.tensor_sub(t_avmv[:, 0, :], t_gc[:], t_lnb[:])
            nc.vector.scalar_tensor_tensor(t_avmv[:, 0, :], t_lnk[:], -0.5,
                                           t_avmv[:, 0, :], op0=ALU.mult, op1=ALU.add)
            nc.vector.scalar_tensor_tensor(t_avmv[:, 1, :], t_lnq[:], -0.5, t_gc[:],
                                           op0=ALU.mult, op1=ALU.add)
            nc.vector.tensor_scalar_add(t_avmv[:, 1, :], t_avmv[:, 1, :],
                                        -0.5 * math.log(DK))
            nc.vector.scalar_tensor_tensor(t_rv[:], t_lnk[:], 0.5, t_gc[:],
                                           op0=ALU.mult, op1=ALU.add)
            yield

            gates = []
            for cc in range(ncps):
                cs = cc * C
                gate_t = gatepool.tile([128, V_CH], BF16, tag="gate")
                for nt in range(2):
                    psg = ps_proj.tile([128, 512], F32, tag="psp")
                    for kt in range(8):
                        nc.tensor.matmul(psg[:], xtr[:, kt, cs:cs + C],
                                         wg_s[:, kt, nt * 512:(nt + 1) * 512],
                                         start=(kt == 0), stop=(kt == 7))
                    nc.scalar.activation(gate_t[:, nt * 512:(nt + 1) * 512], psg[:],
                                         AF.Silu)
                gates.append(gate_t)
                yield
            sc["gates"] = gates

        def gen_scan(sc):
            t0 = sc["t0"]
            qkT, vT = sc["qkT"], sc["vT"]
            t_beta, t_gc = sc["t_beta"], sc["t_gc"]
            t_avmv, t_rv = sc["t_avmv"], sc["t_rv"]
            gates = sc["gates"]
            for cc in range(ncps):
                cs = cc * C
                S_old = S_tiles[chunk_state["idx"] % 2]
                S_new = S_tiles[(chunk_state["idx"] + 1) % 2]

                t_gcc = chpool.tile([8, C], F32, tag="gcc")
                nc.vector.tensor_scalar_mul(t_gcc[:], onesf[0:8, 0:C],
                                            t_gc[:, cs + C - 1:cs + C])
                t_wu = chpool.tile([8, C], F32, tag="wu")
                nc.vector.tensor_tensor(t_wu[:], t_gcc[:], t_rv[:, cs:cs + C],
                                        op=ALU.subtract)

                # cols: 0:8 av | 8:16 mv | 16:24 wU | 24:32 gcC | 32:40 beta | 40:48 rv
                ps_stk = ps_scan.tile([128, 48], F32, tag="ps")
                srcs = [t_avmv[:, 0, cs:cs + C], t_avmv[:, 1, cs:cs + C], t_wu[:],
                        t_gcc[:], t_beta[:, cs:cs + C], t_rv[:, cs:cs + C]]
                for i, src in enumerate(srcs):
                    nc.tensor.transpose(ps_stk[0:C, 8 * i:8 * i + 8], src,
                                        identf[0:8, 0:8])
                stkT = stackpool.tile([128, 48], F32, tag="stkT")
                nc.scalar.activation(stkT[:, 0:32], ps_stk[0:C, 0:32], AF.Exp)
                nc.any.tensor_copy(stkT[:, 32:48], ps_stk[0:C, 32:48])
                nrv = stackpool.tile([128, 8], F32, tag="nrv")
                nc.vector.tensor_scalar_mul(nrv[:], stkT[:, 40:48], -1.0)
                eavT = lambda h: stkT[:, 0 + h:1 + h]
                emvT = lambda h: stkT[:, 8 + h:9 + h]
                ewuT = lambda h: stkT[:, 16 + h:17 + h]
                egcT = lambda h, lo: stkT[lo:lo + 64, 24 + h:25 + h]
                betaT = lambda h: stkT[:, 32 + h:33 + h]
                rvT = lambda h: stkT[:, 40 + h:41 + h]
                yield

                gate_t = gates[cc]
                o_t = opool.tile([128, NH, DV], BF16, tag="ot")

                def head_views(h):
                    lo = (h % 2) * 64
                    kT_h = qkT[:, 4 + h // 2, cs:cs + C][lo:lo + 64, :]
                    qT_h = qkT[:, h // 2, cs:cs + C][lo:lo + 64, :]
                    vT_h = vT[:, h, cs:cs + C]
                    return lo, kT_h, qT_h, vT_h

                bm_l = []
                for h in range(NH):
                    lo, kT_h, qT_h, vT_h = head_views(h)
                    ps_e = ps_scan.tile([128, 2, C], F32, tag="ps")
                    nc.tensor.matmul(ps_e[:, 0, :], kT_h, kT_h, start=True, stop=False,
                                     skip_group_check=True)
                    nc.tensor.matmul(ps_e[:, 1, :], kT_h, qT_h, start=False, stop=True,
                                     skip_group_check=True)
                    ps_r = ps_scan.tile([128, 2, C], F32, tag="ps")
                    nc.tensor.matmul(ps_r[:], sel_s[:, h, :],
                                     t_avmv[:, :, cs:cs + C],
                                     start=True, stop=True)
                    expam = scrpool.tile([128, 2, C], F32, tag="expam")
                    nc.scalar.activation(expam[:], ps_r[:], AF.Exp,
                                         bias=nrv[:, h:h + 1])
                    bm = bmpool.tile([128, 2, C], BF16, tag="bm")
                    nc.vector.tensor_tensor(bm[:], ps_e[:], expam[:], op=ALU.mult)
                    nc.gpsimd.affine_select(bm[:], bm[:], pattern=[[1, 2], [1, C]],
                                            compare_op=ALU.is_gt, fill=0.0, base=0,
                                            channel_multiplier=-1)
                    bm_l.append(bm)
                    if h % 2 == 1:
                        yield

                U_l = [None] * (NH // 2)
                tqs_l = [None] * (NH // 2)
                for hp in range(NH // 2):
                    hA, hB = 2 * hp, 2 * hp + 1
                    U2t = upool.tile([128, 2, DV], BF16, tag="U")
                    tqs2 = tqspool.tile([128, 2, DV], F32, tag="tqs")
                    for i, h in enumerate((hA, hB)):
                        lo, kT_h, qT_h, vT_h = head_views(h)
                        ps_k = ps_scan.tile([128, 2, DV], F32, tag="ps")
                        nc.tensor.matmul(ps_k[:, 0, :], kT_h, s_slice(S_old, h),
                                         start=True, stop=False, skip_group_check=True)
                        nc.tensor.matmul(ps_k[:, 1, :], qT_h, s_slice(S_old, h),
                                         start=False, stop=True, skip_group_check=True)
                        tks = scrpool.tile([128, DV], F32, tag="tks")
                        nc.vector.tensor_scalar_mul(tks[:], ps_k[:, 0, :], eavT(h))
                        nc.vector.tensor_scalar_mul(tqs2[:, i, :], ps_k[:, 1, :],
                                                    emvT(h))
                        ps_vt = ps_b.tile([128, 2, 128], BF16, tag="psb")
                        ps_v = ps_vt[:, 0, :]
                        nc.tensor.transpose(ps_v, vT_h, ident[:])
                        nc.vector.scalar_tensor_tensor(U2t[:, i, :], ps_v, betaT(h),
                                                       tks[:], op0=ALU.mult,
                                                       op1=ALU.subtract)
                    tqs_l[hp] = tqs2
                    U_l[hp] = U2t
                    yield

                Wp_l = [bm_l[h][:, 0, :] for h in range(NH)]
                p = 1
                for lev in range(n_levels):
                    sgn = -1.0 if lev == 0 else 1.0
                    for hp in range(NH // 2):
                        hA, hB = 2 * hp, 2 * hp + 1
                        Uin = U_l[hp]
                        ps_a2 = ps_scan.tile([128, 2, DV], F32, tag="ps")
                        nc.tensor.matmul(ps_a2[:, 0, :], Wp_l[hA], Uin[:, 0, :],
                                         start=True, stop=False, skip_group_check=True)
                        nc.tensor.matmul(ps_a2[:, 1, :], Wp_l[hB], Uin[:, 1, :],
                                         start=False, stop=True, skip_group_check=True)
                        Uout = upool.tile([128, 2, DV], BF16, tag="U")
                        nc.vector.scalar_tensor_tensor(Uout[:], ps_a2[:], sgn, Uin[:],
                                                       op0=ALU.mult, op1=ALU.add)
                        U_l[hp] = Uout
                        if p * 2 < C:
                            ps_tt = ps_b.tile([128, 2, 128], BF16, tag="psb")
                            nc.tensor.transpose(ps_tt[:, 0, :], Wp_l[hA], ident[:])
                            nc.tensor.transpose(ps_tt[:, 1, :], Wp_l[hB], ident[:])
                            WpT2 = wptpool.tile([128, 2, C], BF16, tag="WpT")
                            nc.any.tensor_copy(WpT2[:], ps_tt[:])
                            ps_sq2 = ps_scan.tile([128, 2, C], F32, tag="ps")
                            nc.tensor.matmul(ps_sq2[:, 0, :], WpT2[:, 0, :], Wp_l[hA],
                                             start=True, stop=False,
                                             skip_group_check=True)
                            nc.tensor.matmul(ps_sq2[:, 1, :], WpT2[:, 1, :], Wp_l[hB],
                                             start=False, stop=True,
                                             skip_group_check=True)
                            Wp2 = wppool.tile([128, 2, C], BF16, tag="Wp")
                            nc.any.tensor_copy(Wp2[:], ps_sq2[:])
                            Wp_l[hA] = Wp2[:, 0, :]
                            Wp_l[hB] = Wp2[:, 1, :]
                    p *= 2
                    yield

                for hp in range(NH // 2):
                    hA, hB = 2 * hp, 2 * hp + 1
                    Uin = U_l[hp]
                    tqs2 = tqs_l[hp]
                    ps_o2 = ps_scan.tile([128, 2, DV], F32, tag="ps")
                    nc.tensor.matmul(ps_o2[:, 0, :], bm_l[hA][:, 1, :], Uin[:, 0, :],
                                     start=True, stop=False, skip_group_check=True)
                    nc.tensor.matmul(ps_o2[:, 1, :], bm_l[hB][:, 1, :], Uin[:, 1, :],
                                     start=False, stop=True, skip_group_check=True)
                    nc.vector.tensor_add(tqs2[:], tqs2[:], ps_o2[:])
                    nc.gpsimd.tensor_mul(o_t[:, hA:hA + 2, :], tqs2[:],
                                         gate_t[:, hA * DV:(hB + 1) * DV])
                    for i, h in enumerate((hA, hB)):
                        lo, kT_h, qT_h, vT_h = head_views(h)
                        idh = ident[lo:lo + 64, lo:lo + 64]
                        ps_ktt = ps_b.tile([128, 2, 128], BF16, tag="psb")
                        ps_kt = ps_ktt[:, 0, :]
                        nc.tensor.transpose(ps_kt[:, 0:DK], kT_h, idh)
                        kl2 = scrpool.tile([128, DK], BF16, tag="kl2")
                        nc.vector.tensor_scalar_mul(kl2[:], ps_kt[:, 0:DK], ewuT(h))
                        ps_s = ps_scan.tile([64, DV], F32, tag="ps")
                        nc.tensor.matmul(ps_s[:], kl2[:], Uin[:, i, :], start=True,
                                         stop=True, skip_group_check=True)
                        nc.vector.scalar_tensor_tensor(s_slice(S_new, h),
                                                       s_slice(S_old, h),
                                                       egcT(h, lo), ps_s[:],
                                                       op0=ALU.mult, op1=ALU.add)
                    yield

                nc.sync.dma_start(out_d[t0 + cs:t0 + cs + C, :],
                                  o_t[:].rearrange("p h v -> p (h v)"))
                chunk_state["idx"] += 1

        prev_sc = None
        for s in range(n_super):
            sc = {}
            pg = gen_proj(s, sc)
            sg = gen_scan(prev_sc) if prev_sc is not None else None
            p_done = False
            s_done = sg is None
            while not (p_done and s_done):
                if not p_done:
                    try:
                        next(pg)
                    except StopIteration:
                        p_done = True
                if not s_done:
                    for _ in range(4):
                        try:
                            next(sg)
                        except StopIteration:
                            s_done = True
                            break
            prev_sc = sc
        for _ in gen_scan(prev_sc):
            pass

    nc.compile()
    return nc


def _prep_core_inputs(inputs, core, T=T_FULL):
    b, hg = core // 2, core % 2
    KD = 16 * DK
    VD = 16 * DV
    h0 = hg * NH
    W = inputs["W_in"]
    wq = W[:, h0 * DK:(h0 + NH) * DK]
    wk = W[:, KD + h0 * DK: KD + (h0 + NH) * DK]
    wv = W[:, 2 * KD + h0 * DV: 2 * KD + (h0 + NH) * DV]
    wb = W[:, 2 * KD + VD + h0: 2 * KD + VD + h0 + NH]
    wa = W[:, 2 * KD + VD + 16 + h0: 2 * KD + VD + 16 + h0 + NH]
    ba = np.zeros((D, 40), np.float32)
    ba[:, 0:8] = wb
    ba[:, 32:40] = wa
    wqkv = np.concatenate([wq, wk, wv, ba], axis=1)
    wqkv_t = np.ascontiguousarray(wqkv.reshape(8, 128, IN_COLS))
    wg = inputs["W_gate"][:, h0 * DV:(h0 + NH) * DV]
    wg_t = np.ascontiguousarray(wg.reshape(8, 128, V_CH))
    cw = np.zeros((128, 16, 4), np.float32)
    qw = inputs["q_w"][h0 * DK:(h0 + NH) * DK]
    kw = inputs["k_w"][h0 * DK:(h0 + NH) * DK]
    vw = inputs["v_w"][h0 * DV:(h0 + NH) * DV]
    for t in range(4):
        cw[:, t, :] = qw[t * 128:(t + 1) * 128]
        cw[:, 4 + t, :] = kw[t * 128:(t + 1) * 128]
    for t in range(8):
        cw[:, 8 + t, :] = vw[t * 128:(t + 1) * 128]
    sel = np.zeros((8, 8, 128), np.float32)
    for h in range(8):
        sel[h, h, :] = 1.0
    smallc = np.zeros((8, 4), np.float32)
    smallc[:, 0] = inputs["dt_bias"][h0:h0 + NH]
    smallc[:, 1] = -np.exp(inputs["A_log"][h0:h0 + NH])
    x = np.ascontiguousarray(inputs["hidden_states"][b, :T]).astype(np.float32)
    bf = ml_dtypes.bfloat16
    return {"x": x.astype(bf), "wqkv": wqkv_t.astype(bf), "wg": wg_t.astype(bf),
            "cw": cw, "smallc": smallc, "sel": sel}


def kernel(hidden_states, W_in, q_w, k_w, v_w, dt_bias, A_log, W_gate):
    inputs = dict(hidden_states=np.asarray(hidden_states, np.float32),
                  W_in=np.asarray(W_in, np.float32),
                  q_w=np.asarray(q_w, np.float32), k_w=np.asarray(k_w, np.float32),
                  v_w=np.asarray(v_w, np.float32),
                  dt_bias=np.asarray(dt_bias, np.float32),
                  A_log=np.asarray(A_log, np.float32),
                  W_gate=np.asarray(W_gate, np.float32))
    T = inputs["hidden_states"].shape[1]
    if T not in _CACHE:
        _CACHE[T] = _build(T=T)
    nc = _CACHE[T]
    in_maps = [_prep_core_inputs(inputs, core, T=T) for core in range(8)]
    res = run_bass_kernel_spmd(nc, in_maps, core_ids=list(range(8)))
    out = np.zeros((4, T, 16, 128), np.float32)
    for core in range(8):
        b, hg = core // 2, core % 2
        out[b, :, hg * 8:(hg + 1) * 8, :] = np.asarray(
            res.results[core]["out"], dtype=np.float32).reshape(T, NH, DV)
    return out



# revision 53
# speedup vs baseline: 1.0205x; 1.0205x over previous
"""MixerGatedDeltaNet TRN2 kernel: full-input entry point.

kernel(**inputs) -> np.ndarray [4, 4096, 16, 128] float32.

Sharding: 8 NeuronCores = 4 batches x 2 head-groups. Each core runs the same
Bass program (SPMD) on its (batch, head-group) shard; outputs are gathered.
"""
import math
import sys
from contextlib import ExitStack

import numpy as np

for p in ("/opt/trn_rl_repo",):
    if p not in sys.path:
        sys.path.insert(0, p)

import ml_dtypes
import concourse.bass as bass
import concourse.bacc as bacc
import concourse.tile as tile
from concourse import mybir
from concourse.bass_utils import run_bass_kernel_spmd

dt = mybir.dt
AF = mybir.ActivationFunctionType
ALU = mybir.AluOpType

# Model dims (per core)
D = 1024
NH = 8            # heads per core
DK = 64
DV = 128
QK_CH = NH * DK   # 512
V_CH = NH * DV    # 1024
IN_COLS = 2 * QK_CH + V_CH + 40  # 2088: q 512 | k 512 | v 1024 | b@0:8,a@32:40
EPS = 1e-6
T_FULL = 4096
TS = 512          # super-chunk (projection granularity)
C = 128           # delta-rule chunk length

F32, BF16, F32R = dt.float32, dt.bfloat16, dt.float32r

_CACHE = {}


def _build(T=T_FULL):
    n_super = T // TS
    ncps = TS // C
    n_levels = int(math.log2(C))

    nc = bacc.Bacc("TRN2", target_bir_lowering=False, debug=False, num_devices=8)

    x_d = nc.dram_tensor("x", [T, D], BF16, kind="ExternalInput").ap()
    wqkv_d = nc.dram_tensor("wqkv", [8, 128, IN_COLS], BF16, kind="ExternalInput").ap()
    wg_d = nc.dram_tensor("wg", [8, 128, V_CH], BF16, kind="ExternalInput").ap()
    cw_d = nc.dram_tensor("cw", [128, 16, 4], F32, kind="ExternalInput").ap()
    smallc_d = nc.dram_tensor("smallc", [8, 4], F32, kind="ExternalInput").ap()
    sel_d = nc.dram_tensor("sel", [8, 8, 128], F32, kind="ExternalInput").ap()
    out_d = nc.dram_tensor("out", [T, V_CH], BF16, kind="ExternalOutput").ap()

    with tile.TileContext(nc) as tc, ExitStack() as ctx:
        P = lambda name, bufs, space="SBUF": ctx.enter_context(
            tc.tile_pool(name=name, bufs=bufs, space=space))

        wpool = P("wpool", 1)
        const_pool = P("const", 1)
        xtpool = P("xt", 2)
        qkpool = P("qk", 2)
        vpool = P("v", 2)
        gatepool = P("gate", 6)
        convpool = P("conv", 1)
        halopool = P("halo", 1)
        rowpool = P("row", 2)
        chpool = P("ch", 3)
        stackpool = P("stack", 4)
        bmpool = P("bm", 12)
        upool = P("u", 12)
        wppool = P("wp", 8)
        tqspool = P("tqs", 9)
        scrpool = P("scr", 2)
        wptpool = P("wpt", 3)
        row1pool = P("row1", 1)
        opool = P("o", 2)
        state_pool = P("state", 1)
        ps_proj = P("ps_proj", 2, "PSUM")
        ps_scan = P("ps_scan", 4, "PSUM")
        ps_b = P("ps_b", 2, "PSUM")

        wqkv_s = wpool.tile([128, 8, IN_COLS], BF16)
        nc.sync.dma_start(wqkv_s[:], wqkv_d.rearrange("k p c -> p k c"))
        wg_s = wpool.tile([128, 8, V_CH], BF16)
        nc.sync.dma_start(wg_s[:], wg_d.rearrange("k p c -> p k c"))
        cw_s = const_pool.tile([128, 16, 4], F32)
        nc.sync.dma_start(cw_s[:], cw_d[:])
        smallc_s = const_pool.tile([8, 4], F32)
        nc.sync.dma_start(smallc_s[:], smallc_d[:])
        sel_s = const_pool.tile([8, 8, 128], F32)
        nc.sync.dma_start(sel_s[:], sel_d[:])
        dtb_col = smallc_s[:, 0:1]
        nA_col = smallc_s[:, 1:2]

        identf = const_pool.tile([128, 128], F32)
        ident = const_pool.tile([128, 128], BF16)
        onesf = const_pool.tile([128, 128], F32)
        onesbd = const_pool.tile([128, 2], BF16)
        zeros8 = const_pool.tile([8, C], F32)
        epsc = const_pool.tile([48, 1], F32)
        nc.vector.memset(onesf[:], 1.0)
        nc.vector.memset(zeros8[:], 0.0)
        nc.vector.memset(epsc[:], EPS)
        nc.gpsimd.affine_select(identf[:], onesf[:], pattern=[[-1, 128]],
                                compare_op=ALU.is_equal, fill=0.0, base=0,
                                channel_multiplier=1)
        nc.vector.tensor_copy(ident[:], identf[:])
        nc.vector.memset(onesbd[:], 0.0)
        nc.vector.memset(onesbd[0:64, 0:1], 1.0)
        nc.vector.memset(onesbd[64:128, 1:2], 1.0)

        # per-ti selector stationaries: half-partition sums routed to rows
        # 2ti, 2ti+1 of a shared [16, TS] accumulation bank
        onesbd_all = const_pool.tile([128, 8, 8], BF16)
        nc.vector.memset(onesbd_all[:], 0.0)
        for ti in range(8):
            base = 2 * (ti % 4)
            nc.vector.memset(onesbd_all[0:64, ti, base:base + 1], 1.0)
            nc.vector.memset(onesbd_all[64:128, ti, base + 1:base + 2], 1.0)

        S_a = state_pool.tile([128, 4, DV], BF16, tag="Sa")
        S_b = state_pool.tile([128, 4, DV], BF16, tag="Sb")
        S_tiles = [S_a, S_b]
        nc.vector.memset(S_tiles[0][:], 0.0)
        nc.vector.memset(S_tiles[1][:], 0.0)

        def s_slice(S, h):
            lo = (h % 2) * 64
            return S[lo:lo + 64, h // 2, :]

        diag_cw = wpool.tile([128, 16, 4, 128], BF16)
        for ct in range(16):
            for i in range(4):
                nc.vector.tensor_scalar_mul(diag_cw[:, ct, i, :], ident[:],
                                            cw_s[:, ct, i:i + 1])

        halo = halopool.tile([128, 16, 3], BF16)
        nc.vector.memset(halo[:], 0.0)

        chunk_state = {"idx": 0}

        def gen_proj(s, sc):
            t0 = s * TS
            xt = xtpool.tile([128, 8, TS], BF16)
            nc.sync.dma_start_transpose(xt[:], x_d[t0:t0 + TS, :])
            yield
            xtr = xt[:]

            qkT = qkpool.tile([128, 8, TS], BF16)
            vT = vpool.tile([128, 8, TS], BF16)
            t_beta = rowpool.tile([8, TS], F32, tag="beta")
            t_g = row1pool.tile([8, TS], F32, tag="g")
            t_gc = rowpool.tile([8, TS], F32, tag="gc")
            t_lnb = row1pool.tile([8, TS], F32, tag="lnb")
            t_lnq = row1pool.tile([8, TS], F32, tag="lnq")
            t_lnk = row1pool.tile([8, TS], F32, tag="lnk")
            t_avmv = rowpool.tile([8, 2, TS], F32, tag="avmv")
            t_rv = rowpool.tile([8, TS], F32, tag="rv")
            sc.update(t0=t0, qkT=qkT, vT=vT, t_beta=t_beta, t_gc=t_gc,
                      t_avmv=t_avmv, t_rv=t_rv)

            for ct in range(17):
                c_lo = ct * 128
                n_cols = 128 if ct < 16 else 40
                psp = ps_proj.tile([128, 512], F32, tag="psp")
                for kt in range(8):
                    nc.tensor.matmul(psp[0:n_cols, :],
                                     wqkv_s[:, kt, c_lo:c_lo + n_cols],
                                     xtr[:, kt, :],
                                     start=(kt == 0), stop=(kt == 7))
                if ct < 16:
                    buf = convpool.tile([128, 3 + TS], BF16, tag="cbuf")
                    nc.vector.tensor_copy(buf[:, 0:3], halo[:, ct, :])
                    nc.any.tensor_copy(buf[:, 3:3 + TS], psp[:])
                    nc.vector.tensor_copy(halo[:, ct, :], buf[:, TS:TS + 3])
                    for i in range(4):
                        nc.tensor.matmul(psp[:], diag_cw[:, ct, i, :],
                                         buf[:, i:i + TS],
                                         start=(i == 0), stop=(i == 3))
                    dst = qkT[:, ct, :] if ct < 8 else vT[:, ct - 8, :]
                    nc.scalar.activation(dst, psp[:], AF.Silu)
                else:
                    # beta = 1/(1+e^-b); t_lnb = ln(1+e^-b) = -ln(beta)
                    # g = nA * ln(1+e^(a+dtb))   (nA = -exp(A_log))
                    e3 = row1pool.tile([8, TS], F32, tag="e3")
                    nc.scalar.activation(e3[:], psp[0:8, :], AF.Exp, scale=-1.0)
                    nc.vector.tensor_scalar_add(e3[:], e3[:], 1.0)
                    nc.vector.reciprocal(t_beta[:], e3[:])
                    nc.scalar.activation(t_lnb[:], e3[:], AF.Ln)
                    e2 = row1pool.tile([8, TS], F32, tag="e2")
                    nc.scalar.activation(e2[:], psp[32:40, :], AF.Exp, bias=dtb_col)
                    nc.vector.tensor_scalar_add(e2[:], e2[:], 1.0)
                    nc.scalar.activation(e2[:], e2[:], AF.Ln)
                    nc.vector.tensor_scalar_mul(t_g[:], e2[:], nA_col)
                yield
            for cc in range(ncps):
                nc.vector.tensor_tensor_scan(t_gc[:, cc * C:(cc + 1) * C],
                                             t_g[:, cc * C:(cc + 1) * C],
                                             zeros8[:], 0.0, ALU.add, ALU.add)

            psn_q = ps_scan.tile([8, TS], F32, tag="ps")
            psn_k = ps_scan.tile([8, TS], F32, tag="ps")
            for ti in range(8):
                sq = scrpool.tile([128, TS], BF16, tag="sq")
                nc.gpsimd.tensor_mul(sq[:], qkT[:, ti, :], qkT[:, ti, :])
                psn = psn_q if ti < 4 else psn_k
                nc.tensor.matmul(psn[:], onesbd_all[:, ti, :], sq[:],
                                 start=(ti % 4 == 0), stop=(ti % 4 == 3),
                                 skip_group_check=True)
            nc.scalar.activation(t_lnq[:], psn_q[:], AF.Ln, bias=epsc[0:8, :])
            nc.scalar.activation(t_lnk[:], psn_k[:], AF.Ln, bias=epsc[0:8, :])
            yield

            # av = gc - lnb_pos - 0.5*lnk2 ; mv = gc - 0.5*lnq2 - 0.5*ln(DK)
            # rv = gc + 0.5*lnk2
            nc.vector.tensor_sub(t_avmv[:, 0, :], t_gc[:], t_lnb[:])
            nc.vector.scalar_tensor_tensor(t_avmv[:, 0, :], t_lnk[:], -0.5,
                                           t_avmv[:, 0, :], op0=ALU.mult, op1=ALU.add)
            nc.vector.scalar_tensor_tensor(t_avmv[:, 1, :], t_lnq[:], -0.5, t_gc[:],
                                           op0=ALU.mult, op1=ALU.add)
            nc.vector.tensor_scalar_add(t_avmv[:, 1, :], t_avmv[:, 1, :],
                                        -0.5 * math.log(DK))
            nc.vector.scalar_tensor_tensor(t_rv[:], t_lnk[:], 0.5, t_gc[:],
                                           op0=ALU.mult, op1=ALU.add)
            yield

            gates = []
            for cc in range(ncps):
                cs = cc * C
                gate_t = gatepool.tile([128, V_CH], BF16, tag="gate")
                for nt in range(2):
                    psg = ps_proj.tile([128, 512], F32, tag="psp")
                    for kt in range(8):
                        nc.tensor.matmul(psg[:], xtr[:, kt, cs:cs + C],
                                         wg_s[:, kt, nt * 512:(nt + 1) * 512],
                                         start=(kt == 0), stop=(kt == 7))
                    nc.scalar.activation(gate_t[:, nt * 512:(nt + 1) * 512], psg[:],
                                         AF.Silu)
                gates.append(gate_t)
                yield
            sc["gates"] = gates

        def gen_scan(sc):
            t0 = sc["t0"]
            qkT, vT = sc["qkT"], sc["vT"]
            t_beta, t_gc = sc["t_beta"], sc["t_gc"]
            t_avmv, t_rv = sc["t_avmv"], sc["t_rv"]
            gates = sc["gates"]
            for cc in range(ncps):
                cs = cc * C
                S_old = S_tiles[chunk_state["idx"] % 2]
                S_new = S_tiles[(chunk_state["idx"] + 1) % 2]

                t_gcc = chpool.tile([8, C], F32, tag="gcc")
                nc.vector.tensor_scalar_mul(t_gcc[:], onesf[0:8, 0:C],
                                            t_gc[:, cs + C - 1:cs + C])
                t_wu = chpool.tile([8, C], F32, tag="wu")
                nc.vector.tensor_tensor(t_wu[:], t_gcc[:], t_rv[:, cs:cs + C],
                                        op=ALU.subtract)

                # cols: 0:8 av | 8:16 mv | 16:24 wU | 24:32 gcC | 32:40 beta | 40:48 rv
                ps_stk = ps_scan.tile([128, 48], F32, tag="ps")
                srcs = [t_avmv[:, 0, cs:cs + C], t_avmv[:, 1, cs:cs + C], t_wu[:],
                        t_gcc[:], t_beta[:, cs:cs + C], t_rv[:, cs:cs + C]]
                for i, src in enumerate(srcs):
                    nc.tensor.transpose(ps_stk[0:C, 8 * i:8 * i + 8], src,
                                        identf[0:8, 0:8])
                stkT = stackpool.tile([128, 48], F32, tag="stkT")
                nc.scalar.activation(stkT[:, 0:32], ps_stk[0:C, 0:32], AF.Exp)
                nc.any.tensor_copy(stkT[:, 32:48], ps_stk[0:C, 32:48])
                nrv = stackpool.tile([128, 8], F32, tag="nrv")
                nc.vector.tensor_scalar_mul(nrv[:], stkT[:, 40:48], -1.0)
                eavT = lambda h: stkT[:, 0 + h:1 + h]
                emvT = lambda h: stkT[:, 8 + h:9 + h]
                ewuT = lambda h: stkT[:, 16 + h:17 + h]
                egcT = lambda h, lo: stkT[lo:lo + 64, 24 + h:25 + h]
                betaT = lambda h: stkT[:, 32 + h:33 + h]
                rvT = lambda h: stkT[:, 40 + h:41 + h]
                yield

                gate_t = gates[cc]
                o_t = opool.tile([128, NH, DV], BF16, tag="ot")

                def head_views(h):
                    lo = (h % 2) * 64
                    kT_h = qkT[:, 4 + h // 2, cs:cs + C][lo:lo + 64, :]
                    qT_h = qkT[:, h // 2, cs:cs + C][lo:lo + 64, :]
                    vT_h = vT[:, h, cs:cs + C]
                    return lo, kT_h, qT_h, vT_h

                bm_l = []
                for h in range(NH):
                    lo, kT_h, qT_h, vT_h = head_views(h)
                    ps_e = ps_scan.tile([128, 2, C], F32, tag="ps")
                    nc.tensor.matmul(ps_e[:, 0, :], kT_h, kT_h, start=True, stop=False,
                                     skip_group_check=True)
                    nc.tensor.matmul(ps_e[:, 1, :], kT_h, qT_h, start=False, stop=True,
                                     skip_group_check=True)
                    ps_r = ps_scan.tile([128, 2, C], F32, tag="ps")
                    nc.tensor.matmul(ps_r[:], sel_s[:, h, :],
                                     t_avmv[:, :, cs:cs + C],
                                     start=True, stop=True)
                    expam = scrpool.tile([128, 2, C], F32, tag="expam")
                    nc.scalar.activation(expam[:], ps_r[:], AF.Exp,
                                         bias=nrv[:, h:h + 1])
                    bm = bmpool.tile([128, 2, C], BF16, tag="bm")
                    nc.vector.tensor_tensor(bm[:], ps_e[:], expam[:], op=ALU.mult)
                    nc.gpsimd.affine_select(bm[:], bm[:], pattern=[[1, 2], [1, C]],
                                            compare_op=ALU.is_gt, fill=0.0, base=0,
                                            channel_multiplier=-1)
                    bm_l.append(bm)
                    if h % 2 == 1:
                        yield

                U_l = [None] * (NH // 2)
                tqs_l = [None] * (NH // 2)
                for hp in range(NH // 2):
                    hA, hB = 2 * hp, 2 * hp + 1
                    U2t = upool.tile([128, 2, DV], BF16, tag="U")
                    tqs2 = tqspool.tile([128, 2, DV], F32, tag="tqs")
                    for i, h in enumerate((hA, hB)):
                        lo, kT_h, qT_h, vT_h = head_views(h)
                        ps_k = ps_scan.tile([128, 2, DV], F32, tag="ps")
                        nc.tensor.matmul(ps_k[:, 0, :], kT_h, s_slice(S_old, h),
                                         start=True, stop=False, skip_group_check=True)
                        nc.tensor.matmul(ps_k[:, 1, :], qT_h, s_slice(S_old, h),
                                         start=False, stop=True, skip_group_check=True)
                        tks = scrpool.tile([128, DV], F32, tag="tks")
                        nc.vector.tensor_scalar_mul(tks[:], ps_k[:, 0, :], eavT(h))
                        nc.vector.tensor_scalar_mul(tqs2[:, i, :], ps_k[:, 1, :],
                                                    emvT(h))
                        ps_vt = ps_b.tile([128, 2, 128], BF16, tag="psb")
                        ps_v = ps_vt[:, 0, :]
                        nc.tensor.transpose(ps_v, vT_h, ident[:])
                        nc.vector.scalar_tensor_tensor(U2t[:, i, :], ps_v, betaT(h),
                                                       tks[:], op0=ALU.mult,
                                                       op1=ALU.subtract)
                    tqs_l[hp] = tqs2
                    U_l[hp] = U2t
                    yield

                Wp_l = [bm_l[h][:, 0, :] for h in range(NH)]
                p = 1
                for lev in range(n_levels):
                    sgn = -1.0 if lev == 0 else 1.0
                    for hp in range(NH // 2):
                        hA, hB = 2 * hp, 2 * hp + 1
                        Uin = U_l[hp]
                        ps_a2 = ps_scan.tile([128, 2, DV], F32, tag="ps")
                        nc.tensor.matmul(ps_a2[:, 0, :], Wp_l[hA], Uin[:, 0, :],
                                         start=True, stop=False, skip_group_check=True)
                        nc.tensor.matmul(ps_a2[:, 1, :], Wp_l[hB], Uin[:, 1, :],
                                         start=False, stop=True, skip_group_check=True)
                        Uout = upool.tile([128, 2, DV], BF16, tag="U")
                        nc.vector.scalar_tensor_tensor(Uout[:], ps_a2[:], sgn, Uin[:],
                                                       op0=ALU.mult, op1=ALU.add)
                        U_l[hp] = Uout
                        if p * 2 < C:
                            ps_tt = ps_b.tile([128, 2, 128], BF16, tag="psb")
                            nc.tensor.transpose(ps_tt[:, 0, :], Wp_l[hA], ident[:])
                            nc.tensor.transpose(ps_tt[:, 1, :], Wp_l[hB], ident[:])
                            WpT2 = wptpool.tile([128, 2, C], BF16, tag="WpT")
                            nc.any.tensor_copy(WpT2[:], ps_tt[:])
                            ps_sq2 = ps_scan.tile([128, 2, C], F32, tag="ps")
                            nc.tensor.matmul(ps_sq2[:, 0, :], WpT2[:, 0, :], Wp_l[hA],
                                             start=True, stop=False,
                                             skip_group_check=True)
                            nc.tensor.matmul(ps_sq2[:, 1, :], WpT2[:, 1, :], Wp_l[hB],
                                             start=False, stop=True,
                                             skip_group_check=True)
                            Wp2 = wppool.tile([128, 2, C], BF16, tag="Wp")
                            nc.any.tensor_copy(Wp2[:], ps_sq2[:])
                            Wp_l[hA] = Wp2[:, 0, :]
                            Wp_l[hB] = Wp2[:, 1, :]
                    p *= 2
                    yield

                for hp in range(NH // 2):
                    hA, hB = 2 * hp, 2 * hp + 1
                    Uin = U_l[hp]
                    tqs2 = tqs_l[hp]
                    ps_o2 = ps_scan.tile([128, 2, DV], F32, tag="ps")
                    nc.tensor.matmul(ps_o2[:, 0, :], bm_l[hA][:, 1, :], Uin[:, 0, :],
                                     start=True, stop=False, skip_group_check=True)
                    nc.tensor.matmul(ps_o2[:, 1, :], bm_l[hB][:, 1, :], Uin[:, 1, :],
                                     start=False, stop=True, skip_group_check=True)
                    nc.vector.tensor_add(tqs2[:], tqs2[:], ps_o2[:])
                    nc.gpsimd.tensor_mul(o_t[:, hA:hA + 2, :], tqs2[:],
                                         gate_t[:, hA * DV:(hB + 1) * DV])
                    for i, h in enumerate((hA, hB)):
                        lo, kT_h, qT_h, vT_h = head_views(h)
                        idh = ident[lo:lo + 64, lo:lo + 64]
                        ps_ktt = ps_b.tile([128, 2, 128], BF16, tag="psb")
                        ps_kt = ps_ktt[:, 0, :]
                        nc.tensor.transpose(ps_kt[:, 0:DK], kT_h, idh)
                        kl2 = scrpool.tile([128, DK], BF16, tag="kl2")
                        nc.vector.tensor_scalar_mul(kl2[:], ps_kt[:, 0:DK], ewuT(h))
                        ps_s = ps_scan.tile([64, DV], F32, tag="ps")
                        nc.tensor.matmul(ps_s[:], kl2[:], Uin[:, i, :], start=True,
                                         stop=True, skip_group_check=True)
                        nc.vector.scalar_tensor_tensor(s_slice(S_new, h),
                                                       s_slice(S_old, h),
                                                       egcT(h, lo), ps_s[:],
                                                       op0=ALU.mult, op1=ALU.add)
                    yield

                nc.sync.dma_start(out_d[t0 + cs:t0 + cs + C, :],
                                  o_t[:].rearrange("p h v -> p (h v)"))
                chunk_state["idx"] += 1

        prev_sc = None
        for s in range(n_super):
            sc = {}
            pg = gen_proj(s, sc)
            sg = gen_scan(prev_sc) if prev_sc is not None else None
            p_done = False
            s_done = sg is None
            while not (p_done and s_done):
                if not p_done:
                    try:
                        next(pg)
                    except StopIteration:
                        p_done = True
                if not s_done:
                    for _ in range(4):
                        try:
                            next(sg)
                        except StopIteration:
                            s_done = True
                            break
            prev_sc = sc
        for _ in gen_scan(prev_sc):
            pass

    nc.compile()
    return nc


def _prep_core_inputs(inputs, core, T=T_FULL):
    b, hg = core // 2, core % 2
    KD = 16 * DK
    VD = 16 * DV
    h0 = hg * NH
    W = inputs["W_in"]
    wq = W[:, h0 * DK:(h0 + NH) * DK]
    wk = W[:, KD + h0 * DK: KD + (h0 + NH) * DK]
    wv = W[:, 2 * KD + h0 * DV: 2 * KD + (h0 + NH) * DV]
    wb = W[:, 2 * KD + VD + h0: 2 * KD + VD + h0 + NH]
    wa = W[:, 2 * KD + VD + 16 + h0: 2 * KD + VD + 16 + h0 + NH]
    ba = np.zeros((D, 40), np.float32)
    ba[:, 0:8] = wb
    ba[:, 32:40] = wa
    wqkv = np.concatenate([wq, wk, wv, ba], axis=1)
    wqkv_t = np.ascontiguousarray(wqkv.reshape(8, 128, IN_COLS))
    wg = inputs["W_gate"][:, h0 * DV:(h0 + NH) * DV]
    wg_t = np.ascontiguousarray(wg.reshape(8, 128, V_CH))
    cw = np.zeros((128, 16, 4), np.float32)
    qw = inputs["q_w"][h0 * DK:(h0 + NH) * DK]
    kw = inputs["k_w"][h0 * DK:(h0 + NH) * DK]
    vw = inputs["v_w"][h0 * DV:(h0 + NH) * DV]
    for t in range(4):
        cw[:, t, :] = qw[t * 128:(t + 1) * 128]
        cw[:, 4 + t, :] = kw[t * 128:(t + 1) * 128]
    for t in range(8):
        cw[:, 8 + t, :] = vw[t * 128:(t + 1) * 128]
    sel = np.zeros((8, 8, 128), np.float32)
    for h in range(8):
        sel[h, h, :] = 1.0
    smallc = np.zeros((8, 4), np.float32)
    smallc[:, 0] = inputs["dt_bias"][h0:h0 + NH]
    smallc[:, 1] = -np.exp(inputs["A_log"][h0:h0 + NH])
    x = np.ascontiguousarray(inputs["hidden_states"][b, :T]).astype(np.float32)
    bf = ml_dtypes.bfloat16
    return {"x": x.astype(bf), "wqkv": wqkv_t.astype(bf), "wg": wg_t.astype(bf),
            "cw": cw, "smallc": smallc, "sel": sel}


def kernel(hidden_states, W_in, q_w, k_w, v_w, dt_bias, A_log, W_gate):
    inputs = dict(hidden_states=np.asarray(hidden_states, np.float32),
                  W_in=np.asarray(W_in, np.float32),
                  q_w=np.asarray(q_w, np.float32), k_w=np.asarray(k_w, np.float32),
                  v_w=np.asarray(v_w, np.float32),
                  dt_bias=np.asarray(dt_bias, np.float32),
                  A_log=np.asarray(A_log, np.float32),
                  W_gate=np.asarray(W_gate, np.float32))
    T = inputs["hidden_states"].shape[1]
    if T not in _CACHE:
        _CACHE[T] = _build(T=T)
    nc = _CACHE[T]
    in_maps = [_prep_core_inputs(inputs, core, T=T) for core in range(8)]
    res = run_bass_kernel_spmd(nc, in_maps, core_ids=list(range(8)))
    out = np.zeros((4, T, 16, 128), np.float32)
    for core in range(8):
        b, hg = core // 2, core % 2
        out[b, :, hg * 8:(hg + 1) * 8, :] = np.asarray(
            res.results[core]["out"], dtype=np.float32).reshape(T, NH, DV)
    return out

